# revision 1
# baseline (speedup 1.0000x reference)
"""Trainium2 Bass kernel for capsule attention-routing.

Reference computation (per pixel (b,h,w); 4096 independent problems of
shape [I=32 in-caps, N=32 out-caps, J=16 caps-dim]):
    v[n,j]   = sum_i u[i,n,j]
    cp[i,n]  = sum_j u[i,n,j] * v[n,j] / 4
    c[i,n]   = softmax_n(cp)[i,n] + b[i,n]
    s[n,j]   = sum_i u[i,n,j] * c[i,n]
    out[n,j] = (1 - 1/(exp(|s|_j)+eps)) * s[n,j] / (|s|_j + eps)

Sharding: data-parallel over (batch, h-half): 8 cores x 512 pixels.

Per-core layout: SBUF partitions = (j*8 + il), il = i%8, i = ib*8+il.
All reductions run on the TensorEngine via 0/1/0.25 delta-weight matmuls
(exactly representable -> no weight rounding error):
  v:     contract il (+PSUM-accumulate over ib), broadcast to all rows
  cp:    contract j, pack out partitions (q*32+ib*8+il) with q = p16 pixel blk
  cbc:   broadcast c back over j (K=32 matmuls from c's partition strips)
  s:     contract il (+accum over ib), pack out partitions (q8*16+j), q8 = p8 blk
  norm2: contract j within q8-group, broadcast over group
u streams through PE as float32r; DVE products stored bf16 for 1-cyc/row PE.
Softmax runs without max-subtraction (|cp| <~ 40 is safe in fp32 exp).
EPS=1e-20 is negligible: 1-1/(exp(r)+eps) == 1-exp(-r), 1/(r+eps) == 1/r.
"""

import math
import numpy as np
from contextlib import ExitStack

import concourse.bass as bass
import concourse.bacc as bacc
import concourse.tile as tile
import concourse.mybir as mybir
from concourse.bass_utils import run_bass_kernel_spmd

dt = mybir.dt
AF = mybir.ActivationFunctionType
OP = mybir.AluOpType

B, I, N, J, H, W = 4, 32, 32, 16, 32, 32
HW = H * W
NCORES = 8
PIX = B * HW // NCORES      # 512 pixels per core
BLK = 64                    # pixels per block
P16, P8 = 16, 8
NQ, NQ8 = BLK // P16, BLK // P8   # 4, 8
SCALE = 1.0 / math.sqrt(16.0)     # 0.25

f32, bf16, f32r = dt.float32, dt.bfloat16, dt.float32r
f16 = dt.float16


def _build_weight_arrays():
    il_of = np.arange(128) % 8          # partition -> il
    j_of = np.arange(128) // 8          # partition -> j

    # v-pass: out[(j2,il2)] = sum_il u[(j,il)] for j==j2 (broadcast over il2)
    wv = np.zeros((128, 128), np.float32)
    for p_in in range(128):
        for p_out in range(128):
            if j_of[p_in] == j_of[p_out]:
                wv[p_in, p_out] = 1.0

    # c-reduce: 16 blocks k=q*4+ib: out[q*32+ib*8+il] = SCALE*sum_j w[(j,il)]
    wc = np.zeros((128, 16 * 128), np.float32)
    for q in range(4):
        for ib in range(4):
            k = q * 4 + ib
            for p_in in range(128):
                wc[p_in, k * 128 + q * 32 + ib * 8 + il_of[p_in]] = SCALE

    # c-bcast: row strips q*32..q*32+32 each hold the same [32,128] pattern.
    # in strip: row (ib2*8+il2), col-block ib: col (j*8+il): delta(ib2==ib, il2==il)
    wcb = np.zeros((128, 4 * 128), np.float32)
    for q in range(4):
        for ib in range(4):
            for il in range(8):
                for j in range(16):
                    wcb[q * 32 + ib * 8 + il, ib * 128 + j * 8 + il] = 1.0

    # s-reduce: 8 blocks q8: out[q8*16+j2] = sum_il m[(j,il)] with j==j2
    ws = np.zeros((128, 8 * 128), np.float32)
    for q8 in range(8):
        for p_in in range(128):
            ws[p_in, q8 * 128 + q8 * 16 + j_of[p_in]] = 1.0

    # v-reshape: 8 blocks q8: out[q8*16+j2] = v[(j,0)] with j==j2 (il=0 col)
    wvq = np.zeros((128, 8 * 128), np.float32)
    for q8 in range(8):
        for j in range(16):
            wvq[j * 8 + 0, q8 * 128 + q8 * 16 + j] = 1.0

    # norm2: out[(q8b*16+r)] = sum_j ssq[(q8*16+j)] for q8==q8b
    wn = np.zeros((128, 128), np.float32)
    for p_in in range(128):
        for p_out in range(128):
            if p_in // 16 == p_out // 16:
                wn[p_in, p_out] = 1.0

    return {
        "wv": wv.astype(np.float32),
        "wc": wc.astype(np.dtype(np.float32)).astype("bfloat16")
        if False
        else wc,  # cast handled at upload
        "wcb": wcb,
        "ws": ws,
        "wvq": wvq,
        "wn": wn,
    }


def _b_tile_array(b_np):
    # b_t[q*32+ib*8+il, n*16+p] = b[0, ib*8+il, n, 0,0,0]
    bt = np.zeros((128, N * P16), np.float32)
    bsl = b_np.reshape(I, N)
    for q in range(4):
        for ib in range(4):
            for il in range(8):
                row = q * 32 + ib * 8 + il
                bt[row, :] = np.repeat(bsl[ib * 8 + il, :], P16)
    return bt


def _emit(ctx: ExitStack, tc: tile.TileContext, aps: dict, pix: int, with_b: bool):
    nc = tc.nc
    nblk = pix // BLK
    u_d, o_d = aps["u"], aps["out"]

    # u dram layout: [ib, j, il, blk, n, p]: per-(ib,blk) tile fully contiguous
    u_view = u_d.rearrange("ib j il blk n p -> ib blk (j il) (n p)")

    # constant pool
    pconst = ctx.enter_context(tc.tile_pool(name="const", bufs=1))
    wv_t = pconst.tile([128, 128], f32r, tag="wv")
    wc_t = pconst.tile([128, 16 * 128], f16, tag="wc")
    wcb_t = pconst.tile([32 * 4, 4 * 128], f16, tag="wcb")
    ws_t = pconst.tile([128, 8 * 128], f16, tag="ws")
    wvq_t = pconst.tile([128, 8 * 128], f32r, tag="wvq")
    wn_t = pconst.tile([128, 128], f32r, tag="wn")
    bt_t = pconst.tile([128, N * P16], f32, tag="bt")
    nc.sync.dma_start(wv_t[:], aps["wv"])
    nc.sync.dma_start(wc_t[:], aps["wc"])
    nc.sync.dma_start(wcb_t[:], aps["wcb"])
    nc.sync.dma_start(ws_t[:], aps["ws"])
    nc.sync.dma_start(wvq_t[:], aps["wvq"])
    nc.sync.dma_start(wn_t[:], aps["wn"])
    nc.sync.dma_start(bt_t[:], aps["bt"])

    pu = ctx.enter_context(tc.tile_pool(name="u", bufs=8))
    pw = ctx.enter_context(tc.tile_pool(name="w", bufs=8))
    pm = ctx.enter_context(tc.tile_pool(name="m", bufs=4))
    pvsb = ctx.enter_context(tc.tile_pool(name="vsb", bufs=2))
    psmall = ctx.enter_context(tc.tile_pool(name="small", bufs=2))
    psq = ctx.enter_context(tc.tile_pool(name="sq", bufs=2))

    pvps = ctx.enter_context(tc.tile_pool(name="vps", bufs=2, space="PSUM"))
    pcps = ctx.enter_context(tc.tile_pool(name="cps", bufs=1, space="PSUM"))
    pcb = ctx.enter_context(tc.tile_pool(name="cb", bufs=2, space="PSUM"))
    pvqj = ctx.enter_context(tc.tile_pool(name="vqj", bufs=1, space="PSUM"))
    pspk = ctx.enter_context(tc.tile_pool(name="spk", bufs=2, space="PSUM"))

    for blk in range(nblk):
        # ---- load u tiles: [(j,il), (n, p64)] ----
        T = []
        for ib in range(4):
            t = pu.tile([128, N * BLK], f32r, tag="T")
            nc.sync.dma_start(t[:], u_view[ib, blk])
            T.append(t)

        # ---- v-pass (PE, f32r): v = sum_i u, broadcast over rows ----
        v_sb = pvsb.tile([128, N * BLK], f32r, tag="vsb")
        for st in range(4):
            sl = slice(st * 512, (st + 1) * 512)
            v_ps = pvps.tile([128, 512], f32, tag="vps")
            for ib in range(4):
                nc.tensor.matmul(
                    v_ps[:],
                    wv_t[:],
                    T[ib][:, sl],
                    start=(ib == 0),
                    stop=(ib == 3),
                )
            nc.scalar.copy(v_sb[:, sl], v_ps[:])

        # ---- c-mult (DVE): w = u * v  (store bf16) ----
        Wt = []
        for ib in range(4):
            w = pw.tile([128, N * BLK], f16, tag="w")
            nc.vector.tensor_tensor(
                w[:], T[ib][:].bitcast(f32), v_sb[:].bitcast(f32), op=OP.mult
            )
            Wt.append(w)

        # ---- c-reduce (PE): cp[(q,ib,il), (n,p16)] = SCALE*sum_j w ----
        c_ps = pcps.tile([128, N * P16], f32, tag="cps")
        c_ps_v = c_ps[:].rearrange("P (n p) -> P n p", p=P16)
        for q in range(4):
            for ib in range(4):
                rhs = Wt[ib][:].rearrange("P (n p) -> P n p", p=BLK)[
                    :, :, q * P16 : (q + 1) * P16
                ]
                nc.tensor.matmul(
                    c_ps_v,
                    wc_t[:, (q * 4 + ib) * 128 : (q * 4 + ib + 1) * 128],
                    rhs,
                    start=(q == 0 and ib == 0),
                    stop=(q == 3 and ib == 3),
                    skip_group_check=True,
                )

        # ---- softmax over n (no max-subtraction; |cp| < ~45) ----
        c_e = psmall.tile([128, N * P16], f32, tag="ce")
        nc.scalar.activation(c_e[:], c_ps[:], AF.Exp)
        z = psmall.tile([128, P16], f32, tag="z")
        nc.vector.tensor_reduce(
            z[:],
            c_e[:].rearrange("P (n p) -> P p n", p=P16),
            axis=mybir.AxisListType.X,
            op=OP.add,
        )
        rz = psmall.tile([128, P16], f32, tag="rz")
        nc.vector.reciprocal(rz[:], z[:])
        c_sb = psmall.tile([128, N * P16], f16, tag="csb")
        rz_b = rz[:].rearrange("P (o p) -> P o p", o=1).broadcast_to([128, N, P16])
        if with_b:
            c_f = psmall.tile([128, N * P16], f32, tag="cf")
            nc.vector.tensor_tensor(
                c_f[:].rearrange("P (n p) -> P n p", p=P16),
                c_e[:].rearrange("P (n p) -> P n p", p=P16),
                rz_b,
                op=OP.mult,
            )
            nc.vector.tensor_tensor(c_sb[:], c_f[:], bt_t[:], op=OP.add)
        else:
            nc.vector.tensor_tensor(
                c_sb[:].rearrange("P (n p) -> P n p", p=P16),
                c_e[:].rearrange("P (n p) -> P n p", p=P16),
                rz_b,
                op=OP.mult,
            )

        # ---- c-bcast (PE) + s-mult (DVE) + s-reduce (PE) ----
        spk = pspk.tile([128, N * P8], f32, tag="spk")
        spk_v = spk[:].rearrange("P (n p) -> P n p", p=P8)
        first_s = True
        for ib in range(4):
            for q in range(4):
                cb = pcb.tile([128, N * P16], f32, tag="cb")
                nc.tensor.matmul(
                    cb[:].rearrange("P (n p) -> P n p", p=P16),
                    wcb_t[q * 32 : (q + 1) * 32, ib * 128 : (ib + 1) * 128],
                    c_sb[q * 32 : (q + 1) * 32, :].rearrange(
                        "P (n p) -> P n p", p=P16
                    ),
                    start=True,
                    stop=True,
                    skip_group_check=True,
                    tile_position=(q * 32, 0),
                )
                cb_sb = pm.tile([128, N * P16], f16, tag="cbsb")
                nc.scalar.copy(cb_sb[:], cb[:])
                m = pm.tile([128, N * P16], f16, tag="m")
                nc.vector.tensor_tensor(
                    m[:].rearrange("P (n p) -> P n p", p=P16),
                    T[ib][:].bitcast(f32).rearrange("P (n p) -> P n p", p=BLK)[
                        :, :, q * P16 : (q + 1) * P16
                    ],
                    cb_sb[:].rearrange("P (n p) -> P n p", p=P16),
                    op=OP.mult,
                )
                for k2 in range(2):
                    q8 = 2 * q + k2
                    rhs = m[:].rearrange("P (n p) -> P n p", p=P16)[
                        :, :, k2 * P8 : (k2 + 1) * P8
                    ]
                    nc.tensor.matmul(
                        spk_v,
                        ws_t[:, q8 * 128 : (q8 + 1) * 128],
                        rhs,
                        start=first_s,
                        stop=(ib == 3 and q == 3 and k2 == 1),
                        skip_group_check=True,
                    )
                    first_s = False

        # ssq in f32r: fp16 lacks range for s^2 (HW flushes subnormals),
        # f32r keeps fp32 range at 1 cyc/row on PE.
        ssq = psq.tile([128, N * P8], f32r, tag="ssq")
        nc.scalar.activation(ssq[:], spk[:], AF.Square)
        n2 = pcb.tile([128, N * P8], f32, tag="cb")
        nc.tensor.matmul(n2[:], wn_t[:], ssq[:], start=True, stop=True)
        norm = psq.tile([128, N * P8], f32, tag="norm")
        nc.scalar.activation(norm[:], n2[:], AF.Sqrt)
        en = psq.tile([128, N * P8], f32, tag="en")
        nc.scalar.activation(en[:], norm[:], AF.Exp, scale=-1.0)
        # clamp norm away from 0 so 1/norm can't produce inf (HW underflow
        # of tiny norms -> (en-1)*inf = NaN); out is ~0 there anyway
        norm_c = psq.tile([128, N * P8], f32, tag="normc")
        nc.vector.tensor_scalar(
            norm_c[:], norm[:], 1e-20, None, op0=OP.max
        )
        rn = psq.tile([128, N * P8], f32, tag="rn")
        nc.vector.reciprocal(rn[:], norm_c[:])
        g = psq.tile([128, N * P8], f32, tag="g")
        nc.vector.scalar_tensor_tensor(
            g[:], en[:], 1.0, rn[:], op0=OP.subtract, op1=OP.mult
        )  # g = (en - 1) * rn = -(1-en)/norm
        outt = psq.tile([128, N * P8], f32, tag="outt")
        nc.vector.scalar_tensor_tensor(
            outt[:], spk[:], -1.0, g[:], op0=OP.mult, op1=OP.mult
        )  # (-s) * g = s * (1-en)/norm
        # out DRAM layout mirrors the SBUF tile (host reassembles)
        nc.sync.dma_start(o_d[blk], outt[:])


def round_f32r(x):
    """Round fp32 to the PE's fp32r format: 11-bit mantissa (RNE), low 12 bits 0."""
    b = x.view(np.uint32)
    r = (b + np.uint32(0x7FF) + ((b >> np.uint32(12)) & np.uint32(1))) & np.uint32(
        0xFFFFF000
    )
    return r.view(np.float32)


def encode_u(shard):
    """[I, N, J, pix] -> [ib, J, il, nblk, N, BLK] device layout, fp32r-rounded."""
    pix = shard.shape[-1]
    a = shard.reshape(4, 8, N, J, pix // BLK, BLK)
    return round_f32r(np.ascontiguousarray(a.transpose(0, 3, 1, 4, 2, 5)))


def decode_out(arr, pix):
    """[nblk, 128=(q8,j), N*P8] device layout -> [N, J, pix]."""
    nblk = pix // BLK
    a = arr.reshape(nblk, NQ8, J, N, P8)
    return np.ascontiguousarray(a.transpose(3, 2, 0, 1, 4)).reshape(N, J, pix)


_CACHE = {}


def _patch_act_tables():
    """Force the act-table chooser to natural_log_exp_and_others (which
    contains every function this kernel uses: Copy/Exp/Ln/Square) so only
    ONE table load is emitted instead of per-block set flip-flops.
    Other set entries are kept (emptied) to preserve act_func_set_id
    indices."""
    if getattr(bacc, "_ant_act_tables_patched", False):
        return
    real = bacc.get_activation_tables

    def patched(module_arch):
        tabs = real(module_arch)
        keep = {"natural_log_exp_and_others", "sqrt_and_others"}
        return {
            name: (fns if name in keep else set())
            for name, fns in tabs.items()
        }

    bacc.get_activation_tables = patched
    bacc._ant_act_tables_patched = True


def _get_program(pix, with_b=False):
    key = (pix, with_b)
    if key in _CACHE:
        return _CACHE[key]
    _patch_act_tables()
    nc = bacc.Bacc("TRN2", target_bir_lowering=False, debug=False)
    names = {}
    aps = {}
    aps["u"] = nc.dram_tensor(
        "u", [4, J, 8, pix // BLK, N, BLK], f32r, kind="ExternalInput"
    ).ap()
    wts = _build_weight_arrays()
    aps["wv"] = nc.dram_tensor("wv", [128, 128], f32r, kind="ExternalInput").ap()
    aps["wc"] = nc.dram_tensor("wc", [128, 16 * 128], f16, kind="ExternalInput").ap()
    aps["wcb"] = nc.dram_tensor("wcb", [128, 4 * 128], f16, kind="ExternalInput").ap()
    aps["ws"] = nc.dram_tensor("ws", [128, 8 * 128], f16, kind="ExternalInput").ap()
    aps["wvq"] = nc.dram_tensor(
        "wvq", [128, 8 * 128], f32r, kind="ExternalInput"
    ).ap()
    aps["wn"] = nc.dram_tensor("wn", [128, 128], f32r, kind="ExternalInput").ap()
    aps["bt"] = nc.dram_tensor("bt", [128, N * P16], f32, kind="ExternalInput").ap()
    aps["out"] = nc.dram_tensor(
        "out", [pix // BLK, 128, N * P8], f32, kind="ExternalOutput"
    ).ap()

    with tile.TileContext(nc) as tc:
        with ExitStack() as ctx:
            _emit(ctx, tc, aps, pix, with_b)
    nc.compile()

    _CACHE[key] = (nc, wts)
    return _CACHE[key]


def kernel(u: np.ndarray, b: np.ndarray) -> np.ndarray:
    u = np.asarray(u, dtype=np.float32)
    b = np.asarray(b, dtype=np.float32)
    nc, wts = _get_program(PIX, with_b=bool(np.any(b)))

    import ml_dtypes

    bt = _b_tile_array(b)
    base = {
        "wv": wts["wv"],
        "wc": wts["wc"].astype(np.float16),
        "wcb": wts["wcb"].astype(np.float16),
        "ws": wts["ws"].astype(np.float16),
        "wvq": wts["wvq"],
        "wn": wts["wn"],
        "bt": bt,
    }
    in_maps = []
    for c in range(NCORES):
        bb = c // 2
        h0 = 16 * (c % 2)
        shard = u[bb, :, :, :, h0 : h0 + 16, :].reshape(I, N, J, PIX)
        m = dict(base)
        m["u"] = encode_u(shard)
        in_maps.append(m)

    res = run_bass_kernel_spmd(nc, in_maps, core_ids=list(range(NCORES)))
    out = np.zeros((B, N, J, H, W), np.float32)
    for c in range(NCORES):
        bb = c // 2
        h0 = 16 * (c % 2)
        out[bb, :, :, h0 : h0 + 16, :] = decode_out(
            res.results[c]["out"], PIX
        ).reshape(N, J, 16, W)
    return out



# revision 15
# speedup vs baseline: 1.3830x; 1.3830x over previous
"""Trainium2 Bass kernel for capsule attention-routing.

Reference computation (per pixel; 4096 independent problems of shape
[I=32 in-caps, N=32 out-caps, J=16 caps-dim]):
    v[n,j]   = sum_i u[i,n,j]
    cp[i,n]  = sum_j u[i,n,j] * v[n,j] / 4
    c[i,n]   = softmax_n(cp)[i,n] + b[i,n]
    s[n,j]   = sum_i u[i,n,j] * c[i,n]
    out[n,j] = (1 - exp(-|s|_j)) * s[n,j] / |s|_j

Sharding: data-parallel over (batch, h-half): 8 cores x 512 pixels.

Per-core strategy (dual layout, u streamed twice from HBM as fp16):
  L1 (j-major): partitions (j*8+il), free (ib, n, p64)  [il=i%8, i=ib*8+il]
     - v-pass: PE contracts il (+PSUM accum over ib), broadcast over rows
     - c-mult: DVE 2x-mode fp16 w = u1 * v
     - c-red : PE contracts j via banded 0.25-delta weights ->
               cp[(g*8+il) parts, (ib,n,p4)]  [g = pixel>>2]
  softmax over n on the small cp tile (Act exp f32, Pool z-reduce,
  DVE reciprocal+mult -> c_sb fp16)
  L2 (i-major): partitions (g*8+il), free (ib, n, j, p4)
     - s-mult: DVE 2x-mode m2 = u2 * broadcast_j(c_sb)  (no PE broadcast,
               no PSUM->SBUF copies: c broadcasts via a stride-0 free dim)
     - s-red : PE contracts il within g strips (+accum over ib) ->
               s[(g,x8-replicated) parts, (n8,j,p4)] in 4 nq PSUM banks
  squash: Act square, Pool j-reduce, r = exp(.5*ln(n2)) / rn = exp(-.5*ln n2)
  (single act table: no LoadActFuncSet flips), Pool final multiply.
Softmax runs without max-subtraction (|cp| <~ 45 is safe in fp32 exp).
EPS=1e-20 is negligible: 1-1/(exp(r)+eps) == 1-exp(-r), 1/(r+eps) == 1/r.
"""

import numpy as np
from contextlib import ExitStack

import concourse.bass as bass
import concourse.bacc as bacc
import concourse.tile as tile
import concourse.mybir as mybir
from concourse.bass_utils import run_bass_kernel_spmd

dt = mybir.dt
AF = mybir.ActivationFunctionType
OP = mybir.AluOpType

B, I, N, J, H, W = 4, 32, 32, 16, 32, 32
HW = H * W
NCORES = 8
PIX = B * HW // NCORES      # 512 pixels per core
BLK = 64                    # pixels per block
NBLK = PIX // BLK           # 8
NG = 16                     # pixel groups of 4 per block (g = pixel>>2)
P4 = 4
SCALE = 0.25                # 1/sqrt(16)

f32, bf16, f16 = dt.float32, dt.bfloat16, dt.float16


def _build_weight_arrays():
    il_of = np.arange(128) % 8          # L1 partition -> il is p%8? no: p=(j,il)
    # L1 partitions: p = j*8 + il  -> j = p//8, il = p%8
    j_of = np.arange(128) // 8
    il1 = np.arange(128) % 8

    # v-pass: out[(j2,il2)] = sum_il u[(j,il)] for j==j2 (broadcast over il2)
    wv = np.zeros((128, 128), np.float32)
    for p_in in range(128):
        for p_out in range(128):
            if j_of[p_in] == j_of[p_out]:
                wv[p_in, p_out] = 1.0

    # c-red band: window at offset off(g)=2*(120 - g*8) bytes gives the
    # [128,128] weight mapping (j,il) -> out partition (g*8+il), scaled 0.25.
    # band[(j,il), c] = 0.25 iff c == 120 + il
    wc_band = np.zeros((128, 248), np.float32)
    for p_in in range(128):
        wc_band[p_in, 120 + il1[p_in]] = SCALE

    # s-red band: window at offset off(jq)=2*(6 - jq*2) bytes maps L2
    # partitions (g,il) -> out partition (g*8 + jq*2 + r), r=0,1 replicas.
    # band[(g,il), c] = 1 iff c in (g*8+6, g*8+7)
    ws_band = np.zeros((128, 134), np.float32)
    g_of = np.arange(128) // 8
    for p_in in range(128):
        ws_band[p_in, g_of[p_in] * 8 + 6] = 1.0
        ws_band[p_in, g_of[p_in] * 8 + 7] = 1.0

    # n2: contract the 8 rows of each g strip (each real value appears
    # twice via the r2 replicas -> 0.5)
    wn = np.zeros((128, 128), np.float32)
    for p_in in range(128):
        for p_out in range(128):
            if p_in // 8 == p_out // 8:
                wn[p_in, p_out] = 0.5

    return {"wv": wv, "wc_band": wc_band, "ws_band": ws_band, "wn": wn}


def _b_tile_array(b_np):
    # bt[(g*8+il), (ib, n, p4)] = b[ib*8+il, n]
    bt = np.zeros((128, 4 * N * P4), np.float32)
    bsl = np.asarray(b_np).reshape(I, N)
    for g in range(NG):
        for il in range(8):
            row = g * 8 + il
            for ib in range(4):
                for n in range(N):
                    bt[row, (ib * N + n) * P4 : (ib * N + n + 1) * P4] = bsl[
                        ib * 8 + il, n
                    ]
    return bt


def _emit(ctx: ExitStack, tc: tile.TileContext, aps: dict, with_b: bool):
    nc = tc.nc
    u1_d, u2_d, o_d = aps["u1"], aps["u2"], aps["out"]

    # constants
    pconst = ctx.enter_context(tc.tile_pool(name="const", bufs=1))
    wv_t = pconst.tile([128, 128], f16, tag="wv")
    wcb_t = pconst.tile([128, 248], f16, tag="wcb")
    ws_t = pconst.tile([128, 134], f16, tag="ws")
    wn_t = pconst.tile([128, 128], f16, tag="wn")
    nc.sync.dma_start(wv_t[:], aps["wv"])
    nc.sync.dma_start(wcb_t[:], aps["wc_band"])
    nc.sync.dma_start(ws_t[:], aps["ws_band"])
    nc.sync.dma_start(wn_t[:], aps["wn"])
    bt_t = None
    if with_b:
        bt_t = pconst.tile([128, 4 * N * P4], f32, tag="bt")
        nc.sync.dma_start(bt_t[:], aps["bt"])

    # pools
    pu1 = ctx.enter_context(tc.tile_pool(name="u1", bufs=2))
    pu2 = ctx.enter_context(tc.tile_pool(name="u2", bufs=2))
    pw1 = ctx.enter_context(tc.tile_pool(name="w1", bufs=2))
    pm2 = ctx.enter_context(tc.tile_pool(name="m2", bufs=3))
    pvsb = ctx.enter_context(tc.tile_pool(name="vsb", bufs=2))
    pce = ctx.enter_context(tc.tile_pool(name="ce", bufs=2))
    pcsb = ctx.enter_context(tc.tile_pool(name="csb", bufs=2))
    psq = ctx.enter_context(tc.tile_pool(name="sq", bufs=2))
    pout = ctx.enter_context(tc.tile_pool(name="out", bufs=2))

    pvps = ctx.enter_context(tc.tile_pool(name="vps", bufs=2, space="PSUM"))
    pcps = ctx.enter_context(tc.tile_pool(name="cps", bufs=2, space="PSUM"))
    psps = ctx.enter_context(tc.tile_pool(name="sps", bufs=2, space="PSUM"))
    pnps = ctx.enter_context(tc.tile_pool(name="nps", bufs=2, space="PSUM"))

    for blk in range(NBLK):
        # ---- load u tiles ----
        u1 = pu1.tile([128, 4 * N * BLK], f16, tag="u1")   # [(j,il),(ib,n,p64)]
        nc.sync.dma_start(u1[:], u1_d[blk])
        u2 = pu2.tile([128, 4 * N * J * P4], f16, tag="u2")  # [(g,il),(ib,n,j,p4)]
        nc.sync.dma_start(u2[:], u2_d[blk])

        u1_v = u1[:].rearrange("P (ib n p) -> P ib n p", ib=4, p=BLK)

        # ---- v-pass (PE): v[(j,il-bcast),(n,p64)] = sum_i u1 ----
        v_sb = pvsb.tile([128, N * BLK], f16, tag="vsb")
        v_sb_v = v_sb[:].rearrange("P (n p) -> P n p", p=BLK)
        for st in range(4):
            v_ps = pvps.tile([128, 512], f32, tag="vps")
            v_ps_v = v_ps[:].rearrange("P (n p) -> P n p", p=16)
            for ib in range(4):
                nc.tensor.matmul(
                    v_ps_v,
                    wv_t[:],
                    u1_v[:, ib, :, st * 16 : (st + 1) * 16],
                    start=(ib == 0),
                    stop=(ib == 3),
                )
            nc.scalar.copy(v_sb_v[:, :, st * 16 : (st + 1) * 16], v_ps_v)

        # ---- c-mult (DVE 2x): w1 = u1 * v ----
        w1 = pw1.tile([128, 4 * N * BLK], f16, tag="w1")
        w1_v = w1[:].rearrange("P (ib n p) -> P ib n p", ib=4, p=BLK)
        for ib in range(4):
            nc.vector.tensor_tensor(
                w1_v[:, ib], u1_v[:, ib], v_sb_v, op=OP.mult
            )

        # ---- c-red (PE): cp[(g,il), (ib,n,p4)] = 0.25*sum_j w1 ----
        cp = pcps.tile([128, 4 * N * P4], f32, tag="cp")
        cp_v = cp[:].rearrange("P (ib n p) -> P ib n p", ib=4, p=P4)
        for g in range(NG):
            off = 120 - g * 8
            nc.tensor.matmul(
                cp_v,
                wcb_t[:, off : off + 128],
                w1_v[:, :, :, g * P4 : (g + 1) * P4],
                start=(g == 0),
                stop=(g == NG - 1),
                skip_group_check=True,
            )

        # ---- softmax over n (no max-subtraction) ----
        c_e = pce.tile([128, 4 * N * P4], f32, tag="ce")
        nc.scalar.activation(c_e[:], cp[:], AF.Exp)
        c_e_v = c_e[:].rearrange("P (ib n p) -> P ib n p", ib=4, p=P4)
        z = pcsb.tile([128, 4 * P4], f32, tag="z")
        nc.vector.tensor_reduce(
            z[:].rearrange("P (ib p) -> P ib p", ib=4),
            c_e[:].rearrange("P (ib n p) -> P ib p n", ib=4, p=P4),
            axis=mybir.AxisListType.X,
            op=OP.add,
        )
        rz = pcsb.tile([128, 4 * P4], f32, tag="rz")
        nc.vector.reciprocal(rz[:], z[:])
        rz_b = (
            rz[:]
            .rearrange("P (ib o p) -> P ib o p", ib=4, o=1)
            .broadcast_to([128, 4, N, P4])
        )
        c_sb = pcsb.tile([128, 4 * N * P4], f16, tag="csb")
        c_sb_v = c_sb[:].rearrange("P (ib n p) -> P ib n p", ib=4, p=P4)
        if with_b:
            c_f = pcsb.tile([128, 4 * N * P4], f32, tag="cf")
            nc.gpsimd.tensor_tensor(
                c_f[:].rearrange("P (ib n p) -> P ib n p", ib=4, p=P4),
                c_e_v,
                rz_b,
                op=OP.mult,
            )
            nc.gpsimd.tensor_tensor(c_sb[:], c_f[:], bt_t[:], op=OP.add)
        else:
            nc.gpsimd.tensor_tensor(c_sb_v, c_e_v, rz_b, op=OP.mult)

        # ---- s-phase (L2): m2 = u2 * bcast_j(c_sb); s-red contracts i ----
        # s_all[(g, jq, r2) parts, (nq, m8, jl4, p4)]; j = jq*4 + jl
        u2_v = u2[:].rearrange("P (ib n j p) -> P ib n j p", ib=4, n=N, p=P4)
        s_all = psps.tile([128, 4 * 8 * 4 * P4], f32, tag="sall")
        s_all_v = s_all[:].rearrange(
            "P (q m jl p) -> P q m jl p", q=4, m=8, p=P4
        )
        for ib in range(4):
            m2 = pm2.tile([128, N * J * P4], f16, tag="m2")
            m2_v = m2[:].rearrange("P (n j p) -> P n j p", n=N, p=P4)
            cb = (
                c_sb_v[:, ib]
                .rearrange("P n (o p) -> P n o p", o=1)
                .broadcast_to([128, N, J, P4])
            )
            nc.vector.tensor_tensor(m2_v, u2_v[:, ib], cb, op=OP.mult)
            for nq in range(4):
                for jq in range(4):
                    off = 6 - jq * 2
                    nc.tensor.matmul(
                        s_all_v[:, nq],
                        ws_t[:, off : off + 128],
                        m2_v[
                            :,
                            nq * 8 : (nq + 1) * 8,
                            jq * 4 : (jq + 1) * 4,
                        ],
                        start=(ib == 0 and jq == 0),
                        stop=(ib == 3 and jq == 3),
                        skip_group_check=True,
                    )

        # ---- squash ----
        # ssq = s^2 (bf16 keeps fp32 range; fp16 would flush subnormals)
        ssq = psq.tile([128, 4 * 8 * 4 * P4], bf16, tag="ssq")
        nc.scalar.activation(ssq[:], s_all[:], AF.Square)
        ssq_v = ssq[:].rearrange("P (q m jl p) -> P q m jl p", q=4, m=8, p=P4)
        t1 = psq.tile([128, 4 * 8 * 2 * P4], bf16, tag="t1")
        t1_v = t1[:].rearrange("P (q m jl p) -> P q m jl p", q=4, m=8, p=P4)
        nc.gpsimd.tensor_tensor(
            t1_v, ssq_v[:, :, :, 0:2], ssq_v[:, :, :, 2:4], op=OP.add
        )
        ssq_l = psq.tile([128, 4 * 8 * P4], bf16, tag="ssql")
        nc.gpsimd.tensor_tensor(
            ssq_l[:].rearrange("P (q m p) -> P q m p", q=4, p=P4),
            t1_v[:, :, :, 0],
            t1_v[:, :, :, 1],
            op=OP.add,
        )
        # n2[(g,x8), (nq,m,p4)] = sum_j s^2 via PE partition contraction
        n2 = pnps.tile([128, 4 * 8 * P4], f32, tag="n2")
        nc.tensor.matmul(n2[:], wn_t[:], ssq_l[:], start=True, stop=True)
        # clamp away 0 so ln is finite; out is ~0 there anyway
        n2c = psq.tile([128, 4 * 8 * P4], f32, tag="n2c")
        nc.gpsimd.tensor_scalar(n2c[:], n2[:], 1e-30, None, op0=OP.max)
        lnn = psq.tile([128, 4 * 8 * P4], f32, tag="lnn")
        nc.scalar.activation(lnn[:], n2c[:], AF.Ln)
        # r = exp(.5 ln n2) = |s|; rn = exp(-.5 ln n2) = 1/|s|
        r_t = psq.tile([128, 4 * 8 * P4], f32, tag="r")
        nc.scalar.activation(r_t[:], lnn[:], AF.Exp, scale=0.5)
        rn_t = psq.tile([128, 4 * 8 * P4], f32, tag="rn")
        nc.scalar.activation(rn_t[:], lnn[:], AF.Exp, scale=-0.5)
        en_t = psq.tile([128, 4 * 8 * P4], f32, tag="en")
        nc.scalar.activation(en_t[:], r_t[:], AF.Exp, scale=-1.0)
        g_t = psq.tile([128, 4 * 8 * P4], f32, tag="g")
        nc.vector.scalar_tensor_tensor(
            g_t[:], en_t[:], 1.0, rn_t[:], op0=OP.subtract, op1=OP.mult
        )  # g = (en - 1) / r
        g_b = (
            g_t[:]
            .rearrange("P (q m o p) -> P q m o p", q=4, m=8, o=1)
            .broadcast_to([128, 4, 8, 4, P4])
        )

        outt = pout.tile([128, 4 * 8 * 4 * P4], f16, tag="outt")
        nc.gpsimd.scalar_tensor_tensor(
            outt[:].rearrange("P (q m jl p) -> P q m jl p", q=4, m=8, p=P4),
            s_all_v,
            -1.0,
            g_b,
            op0=OP.mult,
            op1=OP.mult,
        )  # out = (-s) * g = s * (1-en)/r

        # only the r=0 replicas carry data: 64 partitions, stride 2
        nc.sync.dma_start(o_d[blk], outt[::2, :])


def round_f16(x):
    return x.astype(np.float16)


def encode_u1(shard):
    """[I, N, J, pix] -> [blk, (j,il)=128, (ib,n,p64)] fp16."""
    a = shard.reshape(4, 8, N, J, NBLK, BLK)          # ib, il, n, j, blk, p
    # -> blk, j, il, ib, n, p
    return np.ascontiguousarray(
        a.transpose(4, 3, 1, 0, 2, 5)
    ).astype(np.float16)


def encode_u2(shard):
    """[I, N, J, pix] -> [blk, (g,il)=128, (ib,n,j,p4)] fp16."""
    a = shard.reshape(4, 8, N, J, NBLK, NG, P4)       # ib, il, n, j, blk, g, p4
    # -> blk, g, il, ib, n, j, p4
    return np.ascontiguousarray(
        a.transpose(4, 5, 1, 0, 2, 3, 6)
    ).astype(np.float16)


def decode_out(arr):
    """[blk, 64=(g,jq), (nq,m8,jl4,p4)] fp16 -> [N, J, pix] f32.

    n = nq*8+m; j = jq*4+jl; pixel = blk*64 + g*4 + p
    """
    a = arr.astype(np.float32).reshape(NBLK, NG, 4, 4, 8, 4, P4)
    # dims: blk, g, jq, nq, m, jl, p -> (nq,m), (jq,jl), (blk,g,p)
    return np.ascontiguousarray(a.transpose(3, 4, 2, 5, 0, 1, 6)).reshape(
        N, J, PIX
    )


_CACHE = {}


def _patch_act_tables():
    """Keep only natural_log_exp_and_others (Copy/Exp/Ln/Square): every
    function this kernel uses lives in one table, so exactly ONE
    LoadActFuncSet is emitted. Other set entries are kept (emptied) to
    preserve act_func_set_id indices."""
    if getattr(bacc, "_ant_act_tables_patched", False):
        return
    real = bacc.get_activation_tables

    def patched(module_arch):
        tabs = real(module_arch)
        keep = {"natural_log_exp_and_others"}
        return {
            name: (fns if name in keep else set()) for name, fns in tabs.items()
        }

    bacc.get_activation_tables = patched
    bacc._ant_act_tables_patched = True


def _get_program(with_b=False):
    key = with_b
    if key in _CACHE:
        return _CACHE[key]
    _patch_act_tables()
    nc = bacc.Bacc("TRN2", target_bir_lowering=False, debug=False)
    aps = {}
    aps["u1"] = nc.dram_tensor(
        "u1", [NBLK, 128, 4 * N * BLK], f16, kind="ExternalInput"
    ).ap()
    aps["u2"] = nc.dram_tensor(
        "u2", [NBLK, 128, 4 * N * J * P4], f16, kind="ExternalInput"
    ).ap()
    wts = _build_weight_arrays()
    aps["wv"] = nc.dram_tensor("wv", [128, 128], f16, kind="ExternalInput").ap()
    aps["wc_band"] = nc.dram_tensor(
        "wc_band", [128, 248], f16, kind="ExternalInput"
    ).ap()
    aps["ws_band"] = nc.dram_tensor(
        "ws_band", [128, 134], f16, kind="ExternalInput"
    ).ap()
    aps["wn"] = nc.dram_tensor("wn", [128, 128], f16, kind="ExternalInput").ap()
    if with_b:
        aps["bt"] = nc.dram_tensor(
            "bt", [128, 4 * N * P4], f32, kind="ExternalInput"
        ).ap()
    aps["out"] = nc.dram_tensor(
        "out", [NBLK, 64, 4 * 8 * 4 * P4], f16, kind="ExternalOutput"
    ).ap()

    with tile.TileContext(nc) as tc:
        with ExitStack() as ctx:
            _emit(ctx, tc, aps, with_b)
    nc.compile()

    _CACHE[key] = (nc, wts)
    return _CACHE[key]


def kernel(u: np.ndarray, b: np.ndarray) -> np.ndarray:
    u = np.asarray(u, dtype=np.float32)
    b = np.asarray(b, dtype=np.float32)
    with_b = bool(np.any(b))
    nc, wts = _get_program(with_b=with_b)

    base = {
        "wv": wts["wv"].astype(np.float16),
        "wc_band": wts["wc_band"].astype(np.float16),
        "ws_band": wts["ws_band"].astype(np.float16),
        "wn": wts["wn"].astype(np.float16),
    }
    if with_b:
        base["bt"] = _b_tile_array(b)
    in_maps = []
    for c in range(NCORES):
        bb = c // 2
        h0 = 16 * (c % 2)
        shard = u[bb, :, :, :, h0 : h0 + 16, :].reshape(I, N, J, PIX)
        m = dict(base)
        m["u1"] = encode_u1(shard)
        m["u2"] = encode_u2(shard)
        in_maps.append(m)

    res = run_bass_kernel_spmd(nc, in_maps, core_ids=list(range(NCORES)))
    out = np.zeros((B, N, J, H, W), np.float32)
    for c in range(NCORES):
        bb = c // 2
        h0 = 16 * (c % 2)
        out[bb, :, :, h0 : h0 + 16, :] = decode_out(res.results[c]["out"]).reshape(
            N, J, 16, W
        )
    return out
